# revision 12
# baseline (speedup 1.0000x reference)
"""Balanced E/I recurrent layer on 8 Trainium2 NeuronCores.

Contract: kernel(**inputs) takes the FULL inputs (as in reference.setup_inputs)
and returns the FULL output tuple (relu(e_T) [4096,2048] f32, ac/T, bc/T).

Strategy (data-parallel, batch 4096 -> 512 per core, SPMD on 8 cores):
  - All tensors on device are feature-major ([feature, batch_shard]).
  - State kept scaled: s = e / dt  => update s' = (1-dt)*s + net is ONE fused
    DVE op (scalar_tensor_tensor).
  - Matmul operands in fp16 (PE truncates to ~fp22 internally anyway; fp16
    runs at 1 cycle/row vs 4 for fp32), accumulation in fp32 PSUM.
  - Dale's-law scaling split to keep fp16 operands in normal range:
      weights stored as softplus(theta)/64  (negated for I-presynaptic)
      r_e stored as relu(e)/32   (64*32  = 2048 = NE fan-in)
      r_i stored as relu(i)/8    (64*8   = 512  = NI fan-in)
  - softplus computed on device as Ln(Exp(x) + 1) (no HW Softplus table).
  - act/bal cost means: per-(tile,step) partition-column sums via the ACT
    engine's accum_out, reduced on-device to 4 scalars per core; host sums
    across cores and applies the scale/normalization fixups.
"""

import numpy as np

B, D, NE, NI = 4096, 1024, 2048, 512
NCORES = 8
BS = B // NCORES          # 512 batch rows per core
N = BS                    # moving (free) dim of all matmuls
KE, KI, KD = NE // 128, NI // 128, D // 128   # 16, 4, 8 contraction tiles
ME, MI = NE // 128, NI // 128                 # 16, 4 output tiles
DT = 0.2
WSC = 1.0 / 64.0          # weight scale
SE = DT / 32.0            # s_e -> r16_e scale: relu(e)/32 = relu(s*dt)/32
SI = DT / 8.0             # s_i -> r16_i scale
RSQE_FIX = 32.0 * 32.0    # relu(e)^2 = (32*r16)^2
RSQI_FIX = 8.0 * 8.0

_CACHE: dict[int, object] = {}


def _build_nc(T: int):
    import concourse.mybir as mybir
    import concourse.tile as tile
    from concourse import bacc

    AF = mybir.ActivationFunctionType
    f32, f16 = mybir.dt.float32, mybir.dt.float16
    ADD, MULT = mybir.AluOpType.add, mybir.AluOpType.mult

    nc = bacc.Bacc("TRN2", target_bir_lowering=False, debug=False,
                   num_devices=NCORES, name="bei")

    # ---- I/O ----
    xT = nc.dram_tensor("xT", (D, BS), f32, kind="ExternalInput").ap()
    thEET = nc.dram_tensor("thEET", (NE, NE), f32, kind="ExternalInput").ap()
    thEIT = nc.dram_tensor("thEIT", (NI, NE), f32, kind="ExternalInput").ap()
    thIET = nc.dram_tensor("thIET", (NE, NI), f32, kind="ExternalInput").ap()
    thIIT = nc.dram_tensor("thIIT", (NI, NI), f32, kind="ExternalInput").ap()
    wxeT = nc.dram_tensor("wxeT", (D, NE), f32, kind="ExternalInput").ap()
    wxiT = nc.dram_tensor("wxiT", (D, NI), f32, kind="ExternalInput").ap()
    be = nc.dram_tensor("be", (NE,), f32, kind="ExternalInput").ap()
    bi = nc.dram_tensor("bi", (NI,), f32, kind="ExternalInput").ap()
    # net-bias alone (b_e/b_i), subtracted when squaring balance = net - b
    bne = nc.dram_tensor("bne", (NE,), f32, kind="ExternalInput").ap()
    bni = nc.dram_tensor("bni", (NI,), f32, kind="ExternalInput").ap()

    e_out = nc.dram_tensor("e_out", (NE, BS), f32, kind="ExternalOutput").ap()
    sums_out = nc.dram_tensor("sums", (4, 1), f32, kind="ExternalOutput").ap()

    n_r_steps = max(T - 1, 1)  # steps contributing r^2 (j = 0..T-2)

    with tile.TileContext(nc) as tc:
        with (
            tc.tile_pool(name="persist", bufs=1) as pp,
            tc.tile_pool(name="psum", bufs=6, space="PSUM") as psp,
            tc.tile_pool(name="psum4", bufs=1, space="PSUM") as psp4,
        ):
            # ---- persistent state ----
            s_e = pp.tile([128, KE, N], f32, tag="s_e")
            s_i = pp.tile([128, KI, N], f32, tag="s_i")
            r_e = [pp.tile([128, KE, N], f16, tag="r_e0", name="r_e0"),
                   pp.tile([128, KE, N], f16, tag="r_e1", name="r_e1")]
            r_i = pp.tile([128, KI, N], f16, tag="r_i")
            drb_e = pp.tile([128, ME, N], f16, tag="drb_e")
            drb_i = pp.tile([128, MI, N], f16, tag="drb_i")
            acc_re = pp.tile([128, n_r_steps * ME], f32, tag="acc_re")
            acc_ri = pp.tile([128, n_r_steps * MI], f32, tag="acc_ri")
            acc_be = pp.tile([128, T * ME], f32, tag="acc_be")
            acc_bi = pp.tile([128, T * MI], f32, tag="acc_bi")
            acc4 = pp.tile([128, 4], f32, tag="acc4")
            sb4 = pp.tile([4, 1], f32, tag="sb4")
            be_sb = pp.tile([128, ME], f32, tag="be_sb")
            bi_sb = pp.tile([128, MI], f32, tag="bi_sb")
            nbe_sb = pp.tile([128, ME], f32, tag="nbe_sb")
            nbi_sb = pp.tile([128, MI], f32, tag="nbi_sb")

            nc.gpsimd.memset(acc_re[:], 0.0)
            nc.gpsimd.memset(acc_ri[:], 0.0)
            nc.gpsimd.memset(acc_be[:], 0.0)
            nc.gpsimd.memset(acc_bi[:], 0.0)

            nc.sync.dma_start(be_sb[:], be.rearrange("(t p) -> p t", p=128))
            nc.sync.dma_start(bi_sb[:], bi.rearrange("(t p) -> p t", p=128))
            nc.sync.dma_start(nbe_sb[:], bne.rearrange("(t p) -> p t", p=128))
            nc.sync.dma_start(nbi_sb[:], bni.rearrange("(t p) -> p t", p=128))
            nc.vector.tensor_scalar_mul(nbe_sb[:], nbe_sb[:], -1.0)
            nc.vector.tensor_scalar_mul(nbi_sb[:], nbi_sb[:], -1.0)

            # ================= Phase 0: drive matmuls =================
            with (
                tc.tile_pool(name="dhold", bufs=1) as dh,
                tc.tile_pool(name="dstage", bufs=2) as ds_,
            ):
                x16 = dh.tile([128, KD, N], f16, tag="x16")
                wxe16 = dh.tile([128, KD, NE], f16, tag="wxe16")
                wxi16 = dh.tile([128, KD, NI], f16, tag="wxi16")

                for k in range(KD):
                    st = ds_.tile([128, N], f32, tag="stx")
                    nc.sync.dma_start(st[:], xT[k * 128:(k + 1) * 128, :])
                    nc.vector.tensor_copy(x16[:, k, :], st[:])
                for k in range(KD):
                    st = ds_.tile([128, NE], f32, tag="stw")
                    nc.sync.dma_start(st[:], wxeT[k * 128:(k + 1) * 128, :])
                    nc.vector.tensor_copy(wxe16[:, k, :], st[:])
                for k in range(KD):
                    st = ds_.tile([128, NI], f32, tag="stwi")
                    nc.sync.dma_start(st[:], wxiT[k * 128:(k + 1) * 128, :])
                    nc.vector.tensor_copy(wxi16[:, k, :], st[:])

                # drive_e.T + b_e  -> drb_e (fp16), same for I
                for mi in range(ME):
                    p = psp.tile([128, N], f32, tag="p")
                    for k in range(KD):
                        nc.tensor.matmul(p[:], wxe16[:, k, mi * 128:(mi + 1) * 128],
                                         x16[:, k, :], start=(k == 0),
                                         stop=(k == KD - 1))
                    nc.scalar.activation(drb_e[:, mi, :], p[:], AF.Identity,
                                         bias=be_sb[:, mi:mi + 1])
                for mi in range(MI):
                    p = psp.tile([128, N], f32, tag="p")
                    for k in range(KD):
                        nc.tensor.matmul(p[:], wxi16[:, k, mi * 128:(mi + 1) * 128],
                                         x16[:, k, :], start=(k == 0),
                                         stop=(k == KD - 1))
                    nc.scalar.activation(drb_i[:, mi, :], p[:], AF.Identity,
                                         bias=bi_sb[:, mi:mi + 1])

            # ================= Phase 1: Dale weights ==================
            # weights resident; separate pool so it reuses the released
            # drive-pool space (pools reserve their footprint at creation)
            wp_cm = tc.tile_pool(name="wpool", bufs=1)
            wp = wp_cm.__enter__()
            w_ee = wp.tile([128, KE, NE], f16, tag="w_ee")
            w_ei = wp.tile([128, KI, NE], f16, tag="w_ei")
            w_ie = wp.tile([128, KE, NI], f16, tag="w_ie")
            w_ii = wp.tile([128, KI, NI], f16, tag="w_ii")

            with tc.tile_pool(name="tstage", bufs=3) as ts_:
                def softplus_into(theta_ap, w_tile, kt, width, sign):
                    # [128, 512] chunks: W = sign*softplus(theta.T)/64
                    csz = 512
                    for k in range(kt):
                        for c in range(width // csz):
                            st = ts_.tile([128, csz], f32, tag="tchunk")
                            nc.sync.dma_start(
                                st[:], theta_ap[k * 128:(k + 1) * 128,
                                                c * csz:(c + 1) * csz])
                            nc.scalar.activation(st[:], st[:], AF.Exp)
                            nc.scalar.activation(st[:], st[:], AF.Ln, bias=1.0)
                            nc.vector.tensor_scalar_mul(
                                w_tile[:, k, c * csz:(c + 1) * csz], st[:],
                                sign * WSC)

                # small matrices first so step-1 matmuls unblock early
                softplus_into(thIIT, w_ii, KI, NI, -1.0)
                softplus_into(thEIT, w_ei, KI, NE, -1.0)
                softplus_into(thIET, w_ie, KE, NI, 1.0)
                softplus_into(thEET, w_ee, KE, NE, 1.0)

            # ================= Phase 2: recurrence ====================
            with tc.tile_pool(name="fout", bufs=2) as fo:
                for j in range(T):
                    rj = r_e[j % 2]        # r_e tiles read this step
                    rn = r_e[(j + 1) % 2]  # r_e tiles written this step
                    last = (j == T - 1)

                    if j == 0:
                        # s_1 = net_0 = drive(+bias); no matmuls since e_0=0
                        for mt, drb, nb, s_t, acc_b, acc_r, rnx, rsc in (
                            (ME, drb_e, nbe_sb, s_e, acc_be, acc_re, rn, SE),
                            (MI, drb_i, nbi_sb, s_i, acc_bi, acc_ri, r_i, SI),
                        ):
                            for mi in range(mt):
                                p = psp.tile([128, N], f32, tag="p")
                                nc.scalar.activation(
                                    p[:], drb[:, mi, :], AF.Square,
                                    bias=nb[:, mi:mi + 1],
                                    accum_out=acc_b[:, mi:mi + 1])
                                nc.vector.tensor_copy(s_t[:, mi, :],
                                                      drb[:, mi, :])
                                if T > 1:
                                    nc.scalar.activation(rnx[:, mi, :],
                                                         s_t[:, mi, :],
                                                         AF.Relu, scale=rsc)
                                    nc.scalar.activation(
                                        p[:], rnx[:, mi, :], AF.Square,
                                        accum_out=acc_r[:, mi:mi + 1])
                        if last:
                            for mi in range(ME):
                                fo_t = fo.tile([128, N], f32, tag="fo")
                                nc.scalar.activation(fo_t[:], s_e[:, mi, :],
                                                     AF.Relu, scale=DT)
                                nc.sync.dma_start(
                                    e_out[mi * 128:(mi + 1) * 128, :], fo_t[:])
                        continue

                    # E-side groups: psum = W_EE@r_e + W_EI@r_i
                    for mi in range(ME):
                        p = psp.tile([128, N], f32, tag="p")
                        for k in range(KE):
                            nc.tensor.matmul(
                                p[:], w_ee[:, k, mi * 128:(mi + 1) * 128],
                                rj[:, k, :], start=(k == 0), stop=False)
                        for k in range(KI):
                            nc.tensor.matmul(
                                p[:], w_ei[:, k, mi * 128:(mi + 1) * 128],
                                r_i[:, k, :], start=False, stop=(k == KI - 1))
                        # net = rec + drive(+b)   (in-place in PSUM)
                        nc.vector.tensor_tensor(p[:], p[:], drb_e[:, mi, :],
                                                op=ADD)
                        # s' = (1-dt)*s + net     (fused, in-place)
                        nc.vector.scalar_tensor_tensor(
                            s_e[:, mi, :], s_e[:, mi, :], 1.0 - DT, p[:],
                            op0=MULT, op1=ADD)
                        # balance^2 sums; squares scratched into dead psum
                        nc.scalar.activation(
                            p[:], p[:], AF.Square, bias=nbe_sb[:, mi:mi + 1],
                            accum_out=acc_be[:, j * ME + mi:j * ME + mi + 1])
                        if not last:
                            nc.scalar.activation(rn[:, mi, :], s_e[:, mi, :],
                                                 AF.Relu, scale=SE)
                            nc.scalar.activation(
                                p[:], rn[:, mi, :], AF.Square,
                                accum_out=acc_re[:, j * ME + mi:
                                                 j * ME + mi + 1])
                        else:
                            fo_t = fo.tile([128, N], f32, tag="fo")
                            nc.scalar.activation(fo_t[:], s_e[:, mi, :],
                                                 AF.Relu, scale=DT)
                            nc.sync.dma_start(
                                e_out[mi * 128:(mi + 1) * 128, :], fo_t[:])

                    # I-side groups: psum = W_IE@r_e + W_II@r_i
                    psq = []
                    for mi in range(MI):
                        p = psp.tile([128, N], f32, tag="p")
                        for k in range(KE):
                            nc.tensor.matmul(
                                p[:], w_ie[:, k, mi * 128:(mi + 1) * 128],
                                rj[:, k, :], start=(k == 0), stop=False)
                        for k in range(KI):
                            nc.tensor.matmul(
                                p[:], w_ii[:, k, mi * 128:(mi + 1) * 128],
                                r_i[:, k, :], start=False, stop=(k == KI - 1))
                        nc.vector.tensor_tensor(p[:], p[:], drb_i[:, mi, :],
                                                op=ADD)
                        nc.vector.scalar_tensor_tensor(
                            s_i[:, mi, :], s_i[:, mi, :], 1.0 - DT, p[:],
                            op0=MULT, op1=ADD)
                        nc.scalar.activation(
                            p[:], p[:], AF.Square, bias=nbi_sb[:, mi:mi + 1],
                            accum_out=acc_bi[:, j * MI + mi:j * MI + mi + 1])
                        psq.append(p)
                    # r_i is single-buffered: only overwrite it after ALL of
                    # this step's W_II matmuls (which read the old value)
                    # have been emitted.
                    if not last:
                        for mi in range(MI):
                            nc.scalar.activation(r_i[:, mi, :], s_i[:, mi, :],
                                                 AF.Relu, scale=SI)
                            nc.scalar.activation(
                                psq[mi][:], r_i[:, mi, :], AF.Square,
                                accum_out=acc_ri[:, j * MI + mi:
                                                 j * MI + mi + 1])

            # ---- final scalar reduction: 4 partial sums ----
            AX = mybir.AxisListType.X
            nc.vector.reduce_sum(acc4[:, 0:1], acc_re[:], axis=AX)
            nc.vector.reduce_sum(acc4[:, 1:2], acc_ri[:], axis=AX)
            nc.vector.reduce_sum(acc4[:, 2:3], acc_be[:], axis=AX)
            nc.vector.reduce_sum(acc4[:, 3:4], acc_bi[:], axis=AX)
            ones = nc.const_aps.tensor(1.0, (128, 1), f32)
            ps4 = psp4.tile([4, 1], f32, tag="ps4")
            nc.tensor.matmul(ps4[:], acc4[:, 0:4], ones, start=True, stop=True)
            nc.vector.tensor_copy(sb4[:], ps4[:])
            nc.sync.dma_start(sums_out[:], sb4[:])
            wp_cm.__exit__(None, None, None)

    nc.finalize()
    return nc


def _get_nc(T: int):
    nc = _CACHE.get(T)
    if nc is None:
        nc = _build_nc(T)
        _CACHE[T] = nc
    return nc


def run(inputs: dict, trace: bool = False):
    """Run on 8 cores; returns (outputs_tuple, BassKernelResults)."""
    from concourse import bass_utils

    T = int(np.asarray(inputs["T"]))
    g = {k: np.ascontiguousarray(np.asarray(v, dtype=np.float32))
         for k, v in inputs.items() if k != "T"}

    shared = {
        "thEET": np.ascontiguousarray(g["theta_EE"].T),
        "thEIT": np.ascontiguousarray(g["theta_EI"].T),
        "thIET": np.ascontiguousarray(g["theta_IE"].T),
        "thIIT": np.ascontiguousarray(g["theta_II"].T),
        "wxeT": np.ascontiguousarray(g["W_XE_w"].T),
        "wxiT": np.ascontiguousarray(g["W_XI_w"].T),
        # drive_e + b_e is constant across steps: fold both biases into drb
        "be": (g["W_XE_b"] + g["b_e"]).astype(np.float32),
        "bi": (g["W_XI_b"] + g["b_i"]).astype(np.float32),
        # balance = net - b_e: subtract the net-bias alone when squaring
        "bne": g["b_e"],
        "bni": g["b_i"],
    }

    in_maps = []
    for c in range(NCORES):
        m = dict(shared)
        m["xT"] = np.ascontiguousarray(g["x"][c * BS:(c + 1) * BS].T)
        in_maps.append(m)

    nc = _get_nc(T)
    kwargs = {}
    if trace:
        kwargs = dict(trace=True, trace_cores=[0])
    res = bass_utils.run_bass_kernel_spmd(nc, in_maps,
                                          core_ids=list(range(NCORES)),
                                          **kwargs)

    e_full = np.concatenate([res.results[c]["e_out"].T for c in range(NCORES)],
                            axis=0)
    s = np.zeros(4, dtype=np.float64)
    for c in range(NCORES):
        s += res.results[c]["sums"].astype(np.float64).ravel()
    rsqE, rsqI, bsqE, bsqI = s
    ac = (RSQE_FIX * rsqE / (B * NE) + RSQI_FIX * rsqI / (B * NI)) / T
    bc = (bsqE / (B * NE) + bsqI / (B * NI)) / T
    out = (np.ascontiguousarray(e_full, dtype=np.float32),
           np.float32(ac), np.float32(bc))
    return out, res


def kernel(**inputs):
    out, _ = run(inputs)
    return out


# revision 17
# speedup vs baseline: 1366.8627x; 1366.8627x over previous
"""Balanced E/I recurrent layer on 8 Trainium2 NeuronCores.

Contract: kernel(**inputs) takes the FULL inputs (as in reference.setup_inputs)
and returns the FULL output tuple (relu(e_T) [4096,2048] f32, ac/T, bc/T).

Strategy (data-parallel, batch 4096 -> 512 per core, SPMD on 8 cores):
  - All tensors on device are feature-major ([feature, batch_shard]).
  - State kept scaled: s = e / dt  => update s' = (1-dt)*s + net is ONE fused
    DVE op (scalar_tensor_tensor).
  - Matmul operands in fp16 (PE truncates to ~fp22 internally anyway; fp16
    runs at 1 cycle/row vs 4 for fp32), accumulation in fp32 PSUM.
  - Dale's-law scaling split to keep fp16 operands in normal range:
      weights stored as softplus(theta)/64  (negated for I-presynaptic)
      r_e stored as relu(e)/32   (64*32  = 2048 = NE fan-in)
      r_i stored as relu(i)/8    (64*8   = 512  = NI fan-in)
  - softplus computed on device as Ln(Exp(x) + 1) (no HW Softplus table).
  - act/bal cost means: per-(tile,step) partition-column sums via the ACT
    engine's accum_out, reduced on-device to 4 scalars per core; host sums
    across cores and applies the scale/normalization fixups.
"""

import numpy as np

B, D, NE, NI = 4096, 1024, 2048, 512
NCORES = 8
BS = B // NCORES          # 512 batch rows per core
N = BS                    # moving (free) dim of all matmuls
KE, KI, KD = NE // 128, NI // 128, D // 128   # 16, 4, 8 contraction tiles
ME, MI = NE // 128, NI // 128                 # 16, 4 output tiles
DT = 0.2
WSC = 1.0 / 64.0          # weight scale
SE = DT / 32.0            # s_e -> r16_e scale: relu(e)/32 = relu(s*dt)/32
SI = DT / 8.0             # s_i -> r16_i scale
RSQE_FIX = 32.0 * 32.0    # relu(e)^2 = (32*r16)^2
RSQI_FIX = 8.0 * 8.0

_CACHE: dict[int, object] = {}


def _build_nc(T: int):
    import concourse.mybir as mybir
    import concourse.tile as tile
    from concourse import bacc

    AF = mybir.ActivationFunctionType
    f32, f16 = mybir.dt.float32, mybir.dt.float16
    ADD, MULT, BYPASS = (mybir.AluOpType.add, mybir.AluOpType.mult,
                         mybir.AluOpType.bypass)

    # All ACT funcs used here (Exp/Ln/Identity/Relu/Square/Copy) coexist in
    # the 'natural_log_exp_and_others' table. The stock table-load pass picks
    # first-fit per func, thrashing tables (~156 loads x 1.3us). Restrict the
    # choice to the one covering table -> a single load.
    class _Bacc(bacc.Bacc):
        def insert_act_table_loads(self):
            from concourse.hw_specs import get_activation_tables
            import bass_rust as _bass_rust
            has_activation = any(
                isinstance(i, mybir.InstActivation)
                for b in self.main_func.blocks
                for i in b.instructions
            )
            if not has_activation:
                return
            tables = list(get_activation_tables(self.m.arch).items())
            full = "natural_log_exp_and_others"
            if any(n == full for n, _ in tables):
                tables = [(n, (s if n == full else set())) for n, s in tables]
            _bass_rust.insert_act_table_loads(self, tables)

    nc = _Bacc("TRN2", target_bir_lowering=False, debug=False,
               num_devices=NCORES, name="bei")

    # ---- I/O ----
    xT = nc.dram_tensor("xT", (D, BS), f32, kind="ExternalInput").ap()
    thEET = nc.dram_tensor("thEET", (NE, NE), f32, kind="ExternalInput").ap()
    thEIT = nc.dram_tensor("thEIT", (NI, NE), f32, kind="ExternalInput").ap()
    thIET = nc.dram_tensor("thIET", (NE, NI), f32, kind="ExternalInput").ap()
    thIIT = nc.dram_tensor("thIIT", (NI, NI), f32, kind="ExternalInput").ap()
    wxeT = nc.dram_tensor("wxeT", (D, NE), f32, kind="ExternalInput").ap()
    wxiT = nc.dram_tensor("wxiT", (D, NI), f32, kind="ExternalInput").ap()
    be = nc.dram_tensor("be", (NE,), f32, kind="ExternalInput").ap()
    bi = nc.dram_tensor("bi", (NI,), f32, kind="ExternalInput").ap()
    # net-bias alone (b_e/b_i), subtracted when squaring balance = net - b
    bne = nc.dram_tensor("bne", (NE,), f32, kind="ExternalInput").ap()
    bni = nc.dram_tensor("bni", (NI,), f32, kind="ExternalInput").ap()

    e_out = nc.dram_tensor("e_out", (NE, BS), f32, kind="ExternalOutput").ap()
    sums_out = nc.dram_tensor("sums", (4, 1), f32, kind="ExternalOutput").ap()

    n_r_steps = max(T - 1, 1)  # steps contributing r^2 (j = 0..T-2)

    with tile.TileContext(nc) as tc:
        with (
            tc.tile_pool(name="persist", bufs=1) as pp,
            tc.tile_pool(name="psum", bufs=6, space="PSUM") as psp,
            tc.tile_pool(name="psum4", bufs=1, space="PSUM") as psp4,
        ):
            # ---- persistent state ----
            s_e = pp.tile([128, KE, N], f32, tag="s_e")
            s_i = pp.tile([128, KI, N], f32, tag="s_i")
            r_e = [pp.tile([128, KE, N], f16, tag="r_e0", name="r_e0"),
                   pp.tile([128, KE, N], f16, tag="r_e1", name="r_e1")]
            r_i = pp.tile([128, KI, N], f16, tag="r_i")
            drb_e = pp.tile([128, ME, N], f16, tag="drb_e")
            drb_i = pp.tile([128, MI, N], f16, tag="drb_i")
            acc_re = pp.tile([128, n_r_steps * ME], f32, tag="acc_re")
            acc_ri = pp.tile([128, n_r_steps * MI], f32, tag="acc_ri")
            acc_be = pp.tile([128, T * ME], f32, tag="acc_be")
            acc_bi = pp.tile([128, T * MI], f32, tag="acc_bi")
            acc4 = pp.tile([128, 4], f32, tag="acc4")
            sb4 = pp.tile([4, 1], f32, tag="sb4")
            be_sb = pp.tile([128, ME], f32, tag="be_sb")
            bi_sb = pp.tile([128, MI], f32, tag="bi_sb")
            nbe_sb = pp.tile([128, ME], f32, tag="nbe_sb")
            nbi_sb = pp.tile([128, MI], f32, tag="nbi_sb")

            nc.gpsimd.memset(acc_re[:], 0.0)
            nc.gpsimd.memset(acc_ri[:], 0.0)
            nc.gpsimd.memset(acc_be[:], 0.0)
            nc.gpsimd.memset(acc_bi[:], 0.0)

            nc.sync.dma_start(be_sb[:], be.rearrange("(t p) -> p t", p=128))
            nc.sync.dma_start(bi_sb[:], bi.rearrange("(t p) -> p t", p=128))
            nc.sync.dma_start(nbe_sb[:], bne.rearrange("(t p) -> p t", p=128))
            nc.sync.dma_start(nbi_sb[:], bni.rearrange("(t p) -> p t", p=128))
            nc.vector.tensor_scalar_mul(nbe_sb[:], nbe_sb[:], -1.0)
            nc.vector.tensor_scalar_mul(nbi_sb[:], nbi_sb[:], -1.0)

            # ================= Phase 0: drive matmuls =================
            with (
                tc.tile_pool(name="dhold", bufs=1) as dh,
                tc.tile_pool(name="dstage", bufs=2) as ds_,
            ):
                x16 = dh.tile([128, KD, N], f16, tag="x16")
                wxe16 = dh.tile([128, KD, NE], f16, tag="wxe16")
                wxi16 = dh.tile([128, KD, NI], f16, tag="wxi16")

                for k in range(KD):
                    st = ds_.tile([128, N], f32, tag="stx")
                    nc.sync.dma_start(st[:], xT[k * 128:(k + 1) * 128, :])
                    nc.vector.tensor_copy(x16[:, k, :], st[:])
                for k in range(KD):
                    st = ds_.tile([128, NE], f32, tag="stw")
                    nc.sync.dma_start(st[:], wxeT[k * 128:(k + 1) * 128, :])
                    nc.vector.tensor_copy(wxe16[:, k, :], st[:])
                for k in range(KD):
                    st = ds_.tile([128, NI], f32, tag="stwi")
                    nc.sync.dma_start(st[:], wxiT[k * 128:(k + 1) * 128, :])
                    nc.vector.tensor_copy(wxi16[:, k, :], st[:])

                # drive_e.T + b_e  -> drb_e (fp16), same for I
                for mi in range(ME):
                    p = psp.tile([128, N], f32, tag="p")
                    for k in range(KD):
                        nc.tensor.matmul(p[:], wxe16[:, k, mi * 128:(mi + 1) * 128],
                                         x16[:, k, :], start=(k == 0),
                                         stop=(k == KD - 1))
                    nc.scalar.activation(drb_e[:, mi, :], p[:], AF.Identity,
                                         bias=be_sb[:, mi:mi + 1])
                for mi in range(MI):
                    p = psp.tile([128, N], f32, tag="p")
                    for k in range(KD):
                        nc.tensor.matmul(p[:], wxi16[:, k, mi * 128:(mi + 1) * 128],
                                         x16[:, k, :], start=(k == 0),
                                         stop=(k == KD - 1))
                    nc.scalar.activation(drb_i[:, mi, :], p[:], AF.Identity,
                                         bias=bi_sb[:, mi:mi + 1])

            # ================= Phase 1: Dale weights ==================
            # weights resident; separate pool so it reuses the released
            # drive-pool space (pools reserve their footprint at creation)
            wp_cm = tc.tile_pool(name="wpool", bufs=1)
            wp = wp_cm.__enter__()
            w_ee = wp.tile([128, KE, NE], f16, tag="w_ee")
            w_ei = wp.tile([128, KI, NE], f16, tag="w_ei")
            w_ie = wp.tile([128, KE, NI], f16, tag="w_ie")
            w_ii = wp.tile([128, KI, NI], f16, tag="w_ii")

            with tc.tile_pool(name="tstage", bufs=3) as ts_:
                def softplus_into(theta_ap, w_tile, kt, width, sign):
                    # [128, 512] chunks: W = sign*softplus(theta.T)/64
                    csz = 512
                    for k in range(kt):
                        for c in range(width // csz):
                            st = ts_.tile([128, csz], f32, tag="tchunk")
                            nc.sync.dma_start(
                                st[:], theta_ap[k * 128:(k + 1) * 128,
                                                c * csz:(c + 1) * csz])
                            nc.scalar.activation(st[:], st[:], AF.Exp)
                            nc.scalar.activation(st[:], st[:], AF.Ln, bias=1.0)
                            nc.vector.tensor_scalar_mul(
                                w_tile[:, k, c * csz:(c + 1) * csz], st[:],
                                sign * WSC)

                # small matrices first so step-1 matmuls unblock early
                softplus_into(thIIT, w_ii, KI, NI, -1.0)
                softplus_into(thEIT, w_ei, KI, NE, -1.0)
                softplus_into(thIET, w_ie, KE, NI, 1.0)
                softplus_into(thEET, w_ee, KE, NE, 1.0)

            # ================= Phase 2: recurrence ====================
            with tc.tile_pool(name="fout", bufs=2) as fo:
                for j in range(T):
                    rj = r_e[j % 2]        # r_e tiles read this step
                    rn = r_e[(j + 1) % 2]  # r_e tiles written this step
                    last = (j == T - 1)

                    if j == 0:
                        # s_1 = net_0 = drive(+bias); no matmuls since e_0=0
                        for mt, drb, nb, s_t, acc_b, acc_r, rnx, rsc in (
                            (ME, drb_e, nbe_sb, s_e, acc_be, acc_re, rn, SE),
                            (MI, drb_i, nbi_sb, s_i, acc_bi, acc_ri, r_i, SI),
                        ):
                            for mi in range(mt):
                                p = psp.tile([128, N], f32, tag="p")
                                nc.scalar.activation(
                                    p[:], drb[:, mi, :], AF.Square,
                                    bias=nb[:, mi:mi + 1],
                                    accum_out=acc_b[:, mi:mi + 1])
                                nc.vector.tensor_copy(s_t[:, mi, :],
                                                      drb[:, mi, :])
                                if T > 1:
                                    nc.scalar.activation(rnx[:, mi, :],
                                                         s_t[:, mi, :],
                                                         AF.Relu, scale=rsc)
                                    nc.vector.scalar_tensor_tensor(
                                        p[:], rnx[:, mi, :], 0.0,
                                        rnx[:, mi, :], op0=BYPASS, op1=MULT,
                                        accum_out=acc_r[:, mi:mi + 1])
                        if last:
                            for mi in range(ME):
                                fo_t = fo.tile([128, N], f32, tag="fo")
                                nc.scalar.activation(fo_t[:], s_e[:, mi, :],
                                                     AF.Relu, scale=DT)
                                nc.sync.dma_start(
                                    e_out[mi * 128:(mi + 1) * 128, :], fo_t[:])
                        continue

                    # E-side groups: psum = W_EE@r_e + W_EI@r_i
                    for mi in range(ME):
                        p = psp.tile([128, N], f32, tag="p")
                        for k in range(KE):
                            nc.tensor.matmul(
                                p[:], w_ee[:, k, mi * 128:(mi + 1) * 128],
                                rj[:, k, :], start=(k == 0), stop=False)
                        for k in range(KI):
                            nc.tensor.matmul(
                                p[:], w_ei[:, k, mi * 128:(mi + 1) * 128],
                                r_i[:, k, :], start=False, stop=(k == KI - 1))
                        # net = rec + drive(+b)   (in-place in PSUM)
                        nc.vector.tensor_tensor(p[:], p[:], drb_e[:, mi, :],
                                                op=ADD)
                        # s' = (1-dt)*s + net     (fused, in-place)
                        nc.vector.scalar_tensor_tensor(
                            s_e[:, mi, :], s_e[:, mi, :], 1.0 - DT, p[:],
                            op0=MULT, op1=ADD)
                        # balance^2 sums; squares scratched into dead psum
                        nc.scalar.activation(
                            p[:], p[:], AF.Square, bias=nbe_sb[:, mi:mi + 1],
                            accum_out=acc_be[:, j * ME + mi:j * ME + mi + 1])
                        if not last:
                            nc.scalar.activation(rn[:, mi, :], s_e[:, mi, :],
                                                 AF.Relu, scale=SE)
                            nc.vector.scalar_tensor_tensor(
                                p[:], rn[:, mi, :], 0.0, rn[:, mi, :],
                                op0=BYPASS, op1=MULT,
                                accum_out=acc_re[:, j * ME + mi:
                                                 j * ME + mi + 1])
                        else:
                            fo_t = fo.tile([128, N], f32, tag="fo")
                            nc.scalar.activation(fo_t[:], s_e[:, mi, :],
                                                 AF.Relu, scale=DT)
                            nc.sync.dma_start(
                                e_out[mi * 128:(mi + 1) * 128, :], fo_t[:])

                    # I-side groups: psum = W_IE@r_e + W_II@r_i
                    psq = []
                    for mi in range(MI):
                        p = psp.tile([128, N], f32, tag="p")
                        for k in range(KE):
                            nc.tensor.matmul(
                                p[:], w_ie[:, k, mi * 128:(mi + 1) * 128],
                                rj[:, k, :], start=(k == 0), stop=False)
                        for k in range(KI):
                            nc.tensor.matmul(
                                p[:], w_ii[:, k, mi * 128:(mi + 1) * 128],
                                r_i[:, k, :], start=False, stop=(k == KI - 1))
                        nc.vector.tensor_tensor(p[:], p[:], drb_i[:, mi, :],
                                                op=ADD)
                        nc.vector.scalar_tensor_tensor(
                            s_i[:, mi, :], s_i[:, mi, :], 1.0 - DT, p[:],
                            op0=MULT, op1=ADD)
                        nc.scalar.activation(
                            p[:], p[:], AF.Square, bias=nbi_sb[:, mi:mi + 1],
                            accum_out=acc_bi[:, j * MI + mi:j * MI + mi + 1])
                        psq.append(p)
                    # r_i is single-buffered: only overwrite it after ALL of
                    # this step's W_II matmuls (which read the old value)
                    # have been emitted.
                    if not last:
                        for mi in range(MI):
                            nc.scalar.activation(r_i[:, mi, :], s_i[:, mi, :],
                                                 AF.Relu, scale=SI)
                            nc.vector.scalar_tensor_tensor(
                                psq[mi][:], r_i[:, mi, :], 0.0, r_i[:, mi, :],
                                op0=BYPASS, op1=MULT,
                                accum_out=acc_ri[:, j * MI + mi:
                                                 j * MI + mi + 1])

            # ---- final scalar reduction: 4 partial sums ----
            AX = mybir.AxisListType.X
            nc.vector.reduce_sum(acc4[:, 0:1], acc_re[:], axis=AX)
            nc.vector.reduce_sum(acc4[:, 1:2], acc_ri[:], axis=AX)
            nc.vector.reduce_sum(acc4[:, 2:3], acc_be[:], axis=AX)
            nc.vector.reduce_sum(acc4[:, 3:4], acc_bi[:], axis=AX)
            ones = nc.const_aps.tensor(1.0, (128, 1), f32)
            ps4 = psp4.tile([4, 1], f32, tag="ps4")
            nc.tensor.matmul(ps4[:], acc4[:, 0:4], ones, start=True, stop=True)
            nc.vector.tensor_copy(sb4[:], ps4[:])
            nc.sync.dma_start(sums_out[:], sb4[:])
            wp_cm.__exit__(None, None, None)

    nc.finalize()
    return nc


def _get_nc(T: int):
    nc = _CACHE.get(T)
    if nc is None:
        nc = _build_nc(T)
        _CACHE[T] = nc
    return nc


def run(inputs: dict, trace: bool = False):
    """Run on 8 cores; returns (outputs_tuple, BassKernelResults)."""
    from concourse import bass_utils

    T = int(np.asarray(inputs["T"]))
    g = {k: np.ascontiguousarray(np.asarray(v, dtype=np.float32))
         for k, v in inputs.items() if k != "T"}

    shared = {
        "thEET": np.ascontiguousarray(g["theta_EE"].T),
        "thEIT": np.ascontiguousarray(g["theta_EI"].T),
        "thIET": np.ascontiguousarray(g["theta_IE"].T),
        "thIIT": np.ascontiguousarray(g["theta_II"].T),
        "wxeT": np.ascontiguousarray(g["W_XE_w"].T),
        "wxiT": np.ascontiguousarray(g["W_XI_w"].T),
        # drive_e + b_e is constant across steps: fold both biases into drb
        "be": (g["W_XE_b"] + g["b_e"]).astype(np.float32),
        "bi": (g["W_XI_b"] + g["b_i"]).astype(np.float32),
        # balance = net - b_e: subtract the net-bias alone when squaring
        "bne": g["b_e"],
        "bni": g["b_i"],
    }

    in_maps = []
    for c in range(NCORES):
        m = dict(shared)
        m["xT"] = np.ascontiguousarray(g["x"][c * BS:(c + 1) * BS].T)
        in_maps.append(m)

    nc = _get_nc(T)
    kwargs = {}
    if trace:
        kwargs = dict(trace=True, trace_cores=[0])
    res = bass_utils.run_bass_kernel_spmd(nc, in_maps,
                                          core_ids=list(range(NCORES)),
                                          **kwargs)

    e_full = np.concatenate([res.results[c]["e_out"].T for c in range(NCORES)],
                            axis=0)
    s = np.zeros(4, dtype=np.float64)
    for c in range(NCORES):
        s += res.results[c]["sums"].astype(np.float64).ravel()
    rsqE, rsqI, bsqE, bsqI = s
    ac = (RSQE_FIX * rsqE / (B * NE) + RSQI_FIX * rsqI / (B * NI)) / T
    bc = (bsqE / (B * NE) + bsqI / (B * NI)) / T
    out = (np.ascontiguousarray(e_full, dtype=np.float32),
           np.float32(ac), np.float32(bc))
    return out, res


def kernel(**inputs):
    out, _ = run(inputs)
    return out
